# revision 28
# baseline (speedup 1.0000x reference)
"""Trainium2 Bass kernel for a 2-layer GAT (nn_GAT_652835029007).

Sharding: 8 cores = 2 graphs x 4 src-range quarters. Edges are bucketed by
128-node src windows; per window, edges are padded to SLOTS slots of 128.
Per-edge work: one 516-element indirect row gather from a per-layer node
table X[v] = [4x128 head features | alpha_dst x4 | alpha_src x4], a one-hot
segment-reduction matmul into PSUM (attention weight folded into the gathered
rows), and a second tiny matmul for the rowsums. Between layers the
transposed hidden table is AllGather'd within each graph's 4-core group.
"""
import os
import sys

sys.path.insert(0, "/opt/trn_rl_repo")

import numpy as np

import concourse.bass as bass
import concourse.tile as tile
from concourse import mybir
from concourse.masks import make_identity

P = 128
F32 = mybir.dt.float32
I32 = mybir.dt.int32


class Cfg:
    def __init__(self, N, IN_F, E, NPAD, leak=0.2, eps=1e-6, H=4, HID=128, B=2):
        self.N = N            # real node count (incl. padding row 0)
        self.IN_F = IN_F      # input feature dim
        self.E = E            # edges per graph
        self.NPAD = NPAD      # padded node count, = NW * 128, NW % 4 == 0
        self.H = H
        self.HID = HID
        self.B = B
        self.leak = leak
        self.eps = eps
        self.scale = float(np.sqrt(np.float32(H * HID)))
        self.NW = NPAD // P           # total windows
        self.QPG = 4                  # quarters (cores) per graph
        assert self.NW % self.QPG == 0
        self.WPC = self.NW // self.QPG  # windows per core
        self.NPC = NPAD // self.QPG     # node rows per core
        self.TBL = H * HID            # 512 feature cols
        self.ROWE = self.TBL + 2 * H  # 520 dense cols: feats, alpha_dst, alpha_src
        self.AROW = 64                # A-table row: [ad x4, as x4, pad]
        self.NK1 = IN_F // P          # K-tiles layer-1 dense
        self.NK2 = (H * HID) // P     # K-tiles layer-2 dense
        self.TRASH = self.N           # trash row id for pad edges (must be < NPAD)
        # filled at runtime from data:
        self.SLOTS = None
        self.NG = None


FULL = Cfg(N=20001, IN_F=256, E=500000, NPAD=20480)


# --------------------------------------------------------------------------
# host-side prep
# --------------------------------------------------------------------------

def fold_weights(W, a, b, cfg):
    """Build R [K+1, ROWE]: X_aug = [data, 1] @ R.

    cols [h*128:(h+1)*128] = W[h].T ; col AD_OFF+h = W[h].T @ a[h, HID:]
    col AS_OFF+h = W[h].T @ a[h, :HID]; last row = bias contributions.
    """
    H, HID, K = W.shape
    R = np.zeros((K + 1, cfg.ROWE), np.float32)
    for h in range(H):
        R[:K, h * HID:(h + 1) * HID] = W[h].T
        R[K, h * HID:(h + 1) * HID] = b[h]
        ad, asrc = a[h, HID:], a[h, :HID]
        R[:K, cfg.TBL + h] = W[h].T @ ad
        R[K, cfg.TBL + h] = b[h] @ ad
        R[:K, cfg.TBL + H + h] = W[h].T @ asrc
        R[K, cfg.TBL + H + h] = b[h] @ asrc
    return R


def wrap16(flat):
    """dma_gather index format: idx i at partition i%16, col i//16; x8 replicas."""
    n = flat.shape[0]
    w = flat.reshape(n // 16, 16).T.astype(np.int16)
    return np.tile(w, (8, 1))


def bucket_edges(src, dst, cfg):
    """Per (graph,layer): windowed edge buckets.

    Returns (dstb, srel, sabs) each [NW, 128, slots_max] int32 plus the raw
    per-window counts; caller pads slots_max globally.
    """
    order = np.argsort(src, kind="stable")
    ss, ds = src[order], dst[order]
    bnd = np.searchsorted(ss, np.arange(cfg.NW + 1) * P)
    cnts = np.diff(bnd)
    smax = int(np.ceil(cnts.max() / P))
    return (ss, ds, bnd), smax


def fill_buckets(packed, cfg, slots):
    """Per window: int16 wrapped gather indices + int32 srcrel for one-hots.

    Returns dst16g [nw, 128, NG*32] (per-512-idx gather blocks),
    dst16f/src16f [nw, 128, slots*8] (full-window A-table gathers),
    srel [nw, 128, slots].
    """
    ss, ds, bnd = packed
    cap = slots * P
    nw = cfg.NW
    d16f = np.empty((nw, P, cap // 16), np.int16)
    s16f = np.empty((nw, P, cap // 16), np.int16)
    srel = np.empty((nw, P, slots), np.int32)
    dw = np.empty(cap, np.int32)
    sw = np.empty(cap, np.int32)
    for w in range(nw):
        lo, hi = bnd[w], bnd[w + 1]
        n = hi - lo
        dw[:n] = ds[lo:hi]
        dw[n:] = cfg.TRASH
        sw[:n] = ss[lo:hi]
        sw[n:] = cfg.TRASH
        sr = np.full(cap, -1, np.int32)
        sr[:n] = ss[lo:hi] - w * P
        srel[w] = sr.reshape(slots, P).T
        d16f[w] = wrap16(dw)
        s16f[w] = wrap16(sw)
    return d16f, s16f, srel


def prep_inputs(edge_in, edge_out, embed, W1, b1, a1, g1, bn1,
                W2, b2, a2, g2, bn2, Vw, Vb, cfg):
    """Returns (in_maps, cfg) with cfg.SLOTS set."""
    embedT = np.zeros((cfg.IN_F + 1, cfg.NPAD), np.float32)
    embedT[:cfg.IN_F, :cfg.N] = np.asarray(embed, np.float32).T
    embedT[cfg.IN_F, :] = 1.0  # ones row (only row IN_F used for bias bcast)
    R1 = fold_weights(np.asarray(W1, np.float32), np.asarray(a1, np.float32),
                      np.asarray(b1, np.float32), cfg)
    R2 = fold_weights(np.asarray(W2, np.float32), np.asarray(a2, np.float32),
                      np.asarray(b2, np.float32), cfg)
    vwT = np.ascontiguousarray(np.asarray(Vw, np.float32).T)  # [HID, 2]
    vb = np.asarray(Vb, np.float32).reshape(1, -1)

    packs = []
    smax = 1
    for b in range(cfg.B):
        for e in (edge_in, edge_out):
            pk, sm = bucket_edges(np.asarray(e[b][0]), np.asarray(e[b][1]), cfg)
            packs.append(pk)
            smax = max(smax, sm)
    slots = int(np.ceil(smax / 4) * 4)
    cfg.SLOTS = slots
    cfg.NG = slots // 4

    bufs = [fill_buckets(pk, cfg, slots) for pk in packs]  # order: (b0,l1),(b0,l2),(b1,l1),(b1,l2)

    in_maps = []
    for core in range(8):
        g, q = core // cfg.QPG, core % cfg.QPG
        wlo, whi = q * cfg.WPC, (q + 1) * cfg.WPC
        df1, sf1, r1 = bufs[2 * g]
        df2, sf2, r2 = bufs[2 * g + 1]
        in_maps.append(dict(
            embedT=embedT, R1=R1, R2=R2, vwT=vwT, vb=vb,
            g1=np.asarray(g1, np.float32).reshape(1, -1),
            bn1=np.asarray(bn1, np.float32).reshape(1, -1),
            g2=np.asarray(g2, np.float32).reshape(1, -1),
            bn2=np.asarray(bn2, np.float32).reshape(1, -1),
            df1=df1[wlo:whi], sf1=sf1[wlo:whi], srel1=r1[wlo:whi],
            df2=df2[wlo:whi], sf2=sf2[wlo:whi], srel2=r2[wlo:whi],
        ))
    return in_maps


# --------------------------------------------------------------------------
# device kernel
# --------------------------------------------------------------------------

def build_gat(tc, outs, ins, cfg):
    nc = tc.nc
    H, HID, TBL, ROWE = cfg.H, cfg.HID, cfg.TBL, cfg.ROWE
    SLOTS, NG, WPC, NW, NPC = cfg.SLOTS, cfg.NG, cfg.WPC, cfg.NW, cfg.NPC
    inv_scale = 1.0 / cfg.scale

    with tc.tile_pool(name="consts", bufs=1) as cp, \
         tc.tile_pool(name="dram", bufs=1, space="DRAM") as dram, \
         tc.tile_pool(name="work", bufs=3) as wp, \
         tc.tile_pool(name="gpool", bufs=min(NG + 2, 5)) as gp, \
         tc.tile_pool(name="apool", bufs=2) as app, \
         tc.tile_pool(name="spool", bufs=2) as sp, \
         tc.tile_pool(name="psum", bufs=2, space="PSUM") as pp:

        # ---- constants ----
        ident = cp.tile([P, P], F32)
        make_identity(nc, ident[:])
        iota = cp.tile([P, P], F32)
        nc.gpsimd.iota(iota[:], pattern=[[1, P]], base=0, channel_multiplier=0,
                       allow_small_or_imprecise_dtypes=True)
        r1_sb = []
        for k in range(cfg.NK1):
            t = cp.tile([P, ROWE], F32, name=f"r1_{k}")
            nc.sync.dma_start(out=t[:], in_=ins["R1"][k * P:(k + 1) * P, :])
            r1_sb.append(t)
        bias1 = cp.tile([P, ROWE], F32)
        nc.gpsimd.dma_start(out=bias1[:],
                            in_=ins["R1"][cfg.IN_F:cfg.IN_F + 1, :].to_broadcast([P, ROWE]))
        r2_sb = []
        for k in range(cfg.NK2):
            t = cp.tile([P, ROWE], F32, name=f"r2_{k}")
            nc.sync.dma_start(out=t[:], in_=ins["R2"][k * P:(k + 1) * P, :])
            r2_sb.append(t)
        bias2 = cp.tile([P, ROWE], F32)
        nc.gpsimd.dma_start(out=bias2[:],
                            in_=ins["R2"][TBL:TBL + 1, :].to_broadcast([P, ROWE]))
        g1b = cp.tile([P, TBL], F32)
        nc.gpsimd.dma_start(out=g1b[:], in_=ins["g1"][0:1, :].to_broadcast([P, TBL]))
        bn1b = cp.tile([P, TBL], F32)
        nc.gpsimd.dma_start(out=bn1b[:], in_=ins["bn1"][0:1, :].to_broadcast([P, TBL]))
        g2b = cp.tile([P, HID], F32)
        nc.gpsimd.dma_start(out=g2b[:], in_=ins["g2"][0:1, :].to_broadcast([P, HID]))
        bn2b = cp.tile([P, HID], F32)
        nc.gpsimd.dma_start(out=bn2b[:], in_=ins["bn2"][0:1, :].to_broadcast([P, HID]))
        vwT = cp.tile([P, 2], F32)
        nc.sync.dma_start(out=vwT[:], in_=ins["vwT"][:, :])
        vbb = cp.tile([P, 2], F32)
        nc.gpsimd.dma_start(out=vbb[:], in_=ins["vb"][0:1, :].to_broadcast([P, 2]))

        # ---- DRAM scratch ----
        X1 = dram.tile([cfg.NPAD, TBL], F32)
        A1 = dram.tile([cfg.NPAD, cfg.AROW], F32)
        X2 = dram.tile([cfg.NPAD, TBL], F32)
        A2 = dram.tile([cfg.NPAD, cfg.AROW], F32)
        hT_loc = dram.tile([TBL, NPC], F32)
        hT_ag = dram.tile([cfg.QPG * TBL, NPC], F32)

        # ---- dense phase ----
        def dense(xd, ad, lhsT_fn, rhs_tiles, bias):
            nk = len(rhs_tiles)
            for t in range(NW):
                ps = pp.tile([P, ROWE], F32, tag="big", name="dps")
                for k in range(nk):
                    lt = wp.tile([P, P], F32, tag="lhsT", name="lt")
                    nc.sync.dma_start(out=lt[:], in_=lhsT_fn(k, t))
                    nc.tensor.matmul(ps[:, 0:512], lhsT=lt[:],
                                     rhs=rhs_tiles[k][:, 0:512],
                                     start=(k == 0), stop=(k == nk - 1))
                    nc.tensor.matmul(ps[:, 512:ROWE], lhsT=lt[:],
                                     rhs=rhs_tiles[k][:, 512:ROWE],
                                     start=(k == 0), stop=(k == nk - 1))
                ev = wp.tile([P, TBL], F32, tag="evac", name="ev")
                nc.vector.tensor_tensor(out=ev[:], in0=ps[:, 0:TBL],
                                        in1=bias[:, 0:TBL],
                                        op=mybir.AluOpType.add)
                nc.sync.dma_start(out=xd[t * P:(t + 1) * P, :], in_=ev[:])
                eva = wp.tile([P, cfg.AROW], F32, tag="evaca", name="eva")
                nc.vector.memset(eva[:], 0.0)
                nc.vector.tensor_tensor(out=eva[:, 0:2 * H], in0=ps[:, TBL:ROWE],
                                        in1=bias[:, TBL:ROWE],
                                        op=mybir.AluOpType.add)
                nc.sync.dma_start(out=ad[t * P:(t + 1) * P, :], in_=eva[:])

        # ---- edge phase ----
        def edge(layer, xd, ad, dfi, sfi, sreli):
            NIW = SLOTS * P  # indices per window
            for w in range(WPC):
                df16 = wp.tile([P, NIW // 16], mybir.dt.int16, tag="df16", name="df16")
                nc.sync.dma_start(out=df16[:], in_=dfi[w])
                sf16 = wp.tile([P, NIW // 16], mybir.dt.int16, tag="sf16", name="sf16")
                nc.sync.dma_start(out=sf16[:], in_=sfi[w])
                srel_t = wp.tile([P, SLOTS], I32, tag="srel", name="srel_t")
                nc.sync.dma_start(out=srel_t[:], in_=sreli[w])
                srel_f = wp.tile([P, SLOTS], F32, tag="srelf", name="srel_f")
                nc.vector.tensor_copy(srel_f[:], srel_t[:])

                S = sp.tile([P, SLOTS, P], F32, tag="S", name="S")
                in0 = bass.AP(tensor=iota[:].tensor, offset=iota[:].offset,
                              ap=[iota[:].ap[0], [0, SLOTS], [1, P]])
                sr = srel_f[:]
                in1 = bass.AP(tensor=sr.tensor, offset=sr.offset,
                              ap=[sr.ap[0], [1, SLOTS], [0, P]])
                nc.vector.tensor_tensor(out=S[:], in0=in0, in1=in1,
                                        op=mybir.AluOpType.is_equal)

                Ad = app.tile([P, SLOTS, cfg.AROW], F32, tag="Ad", name="Ad")
                As = app.tile([P, SLOTS, cfg.AROW], F32, tag="As", name="As")
                for g in range(NG):
                    isl = slice(g * 32, (g + 1) * 32)
                    osl = slice(4 * g, 4 * (g + 1))
                    nc.gpsimd.dma_gather(Ad[:, osl, :], ad[:], df16[:, isl],
                                         512, 512, cfg.AROW)
                    nc.gpsimd.dma_gather(As[:, osl, :], ad[:], sf16[:, isl],
                                         512, 512, cfg.AROW)

                Gs = []
                for g in range(NG):
                    G = gp.tile([P, 4, TBL], F32, tag="G", name="G")
                    nc.gpsimd.dma_gather(G[:], xd[:],
                                         df16[:, g * 32:(g + 1) * 32],
                                         512, 512, TBL)
                    Gs.append(G)

                sig = wp.tile([P, SLOTS, H], F32, tag="sig", name="sig")
                nc.vector.tensor_tensor(out=sig[:], in0=Ad[:, :, 0:H],
                                        in1=As[:, :, H:2 * H],
                                        op=mybir.AluOpType.add)
                t1 = wp.tile([P, SLOTS, H], F32, tag="t1", name="t1")
                nc.vector.tensor_scalar_mul(t1[:], sig[:], inv_scale)
                t2 = wp.tile([P, SLOTS, H], F32, tag="t2", name="t2")
                nc.vector.tensor_scalar_mul(t2[:], t1[:], cfg.leak)
                t3 = wp.tile([P, SLOTS, H], F32, tag="t3", name="t3")
                nc.vector.tensor_tensor(out=t3[:], in0=t1[:], in1=t2[:],
                                        op=mybir.AluOpType.max)
                wexp = wp.tile([P, SLOTS, H], F32, tag="wexp", name="wexp")
                nc.scalar.activation(wexp[:], t3[:], mybir.ActivationFunctionType.Exp)

                for g in range(NG):
                    gv = Gs[g][:, :, :].rearrange("p s (h c) -> p s h c", c=HID)
                    wb = wexp[:, 4 * g:4 * (g + 1), :]
                    w4 = bass.AP(tensor=wb.tensor, offset=wb.offset,
                                 ap=[*wb.ap, [0, HID]])
                    nc.vector.tensor_tensor(out=gv, in0=gv, in1=w4,
                                            op=mybir.AluOpType.mult)

                hp = pp.tile([P, TBL], F32, tag="big", name="hp")
                rs = pp.tile([P, H], F32, tag="small", name="rs")
                for s in range(SLOTS):
                    g, j = divmod(s, 4)
                    st, sp_ = (s == 0), (s == SLOTS - 1)
                    nc.tensor.matmul(hp[:], lhsT=S[:, s, :],
                                     rhs=Gs[g][:, j, :], start=st, stop=sp_)
                    nc.tensor.matmul(rs[:], lhsT=S[:, s, :],
                                     rhs=wexp[:, s, :], start=st, stop=sp_)

                iz = wp.tile([P, H], F32, tag="iz", name="iz")
                nc.vector.tensor_scalar(out=iz[:], in0=rs[:], scalar1=0.0,
                                        scalar2=None, op0=mybir.AluOpType.is_equal)
                rsf = wp.tile([P, H], F32, tag="rsf", name="rsf")
                nc.vector.tensor_tensor(out=rsf[:], in0=rs[:], in1=iz[:],
                                        op=mybir.AluOpType.add)
                rcp = wp.tile([P, H], F32, tag="rcp", name="rcp")
                nc.vector.reciprocal(rcp[:], rsf[:])

                if layer == 1:
                    hn = wp.tile([P, TBL], F32, tag="hn", name="hn", bufs=2)
                    for h in range(H):
                        nc.scalar.activation(hn[:, h * HID:(h + 1) * HID],
                                             hp[:, h * HID:(h + 1) * HID],
                                             mybir.ActivationFunctionType.Copy,
                                             scale=rcp[:, h:h + 1])
                    D = TBL
                else:
                    nc.vector.tensor_scalar_mul(rcp[:], rcp[:], 1.0 / H)
                    hn = wp.tile([P, HID], F32, tag="hn2", name="hn")
                    nc.scalar.activation(hn[:], hp[:, 0:HID],
                                         mybir.ActivationFunctionType.Copy,
                                         scale=rcp[:, 0:1])
                    for h in range(1, H):
                        th = wp.tile([P, HID], F32, tag="th", name="th")
                        nc.scalar.activation(th[:], hp[:, h * HID:(h + 1) * HID],
                                             mybir.ActivationFunctionType.Copy,
                                             scale=rcp[:, h:h + 1])
                        nc.vector.tensor_tensor(out=hn[:], in0=hn[:], in1=th[:],
                                                op=mybir.AluOpType.add)
                    D = HID

                # layernorm (torch-style: unbiased std, eps on std)
                stats = wp.tile([P, nc.vector.BN_STATS_DIM], F32, tag="bst", name="stats")
                nc.vector.bn_stats(out=stats[:], in_=hn[:])
                mv = wp.tile([P, nc.vector.BN_AGGR_DIM], F32, tag="mv", name="mv")
                nc.vector.bn_aggr(out=mv[:], in_=stats[:])
                sd = wp.tile([P, 1], F32, tag="sd", name="sd")
                nc.scalar.activation(sd[:], mv[:, 1:2],
                                     mybir.ActivationFunctionType.Sqrt,
                                     scale=float(D) / (D - 1))
                nc.vector.tensor_scalar_add(sd[:], sd[:], cfg.eps)
                rstd = wp.tile([P, 1], F32, tag="rstd", name="rstd")
                nc.vector.reciprocal(rstd[:], sd[:])
                xn = wp.tile([P, D], F32, tag=f"xn{layer}", name="xn", bufs=2)
                nc.vector.tensor_scalar(out=xn[:], in0=hn[:],
                                        scalar1=mv[:, 0:1], scalar2=rstd[:, 0:1],
                                        op0=mybir.AluOpType.subtract,
                                        op1=mybir.AluOpType.mult)
                gmb = g1b if layer == 1 else g2b
                bnb = bn1b if layer == 1 else bn2b
                nc.vector.tensor_tensor(out=xn[:], in0=xn[:], in1=gmb[:, 0:D],
                                        op=mybir.AluOpType.mult)
                nc.vector.tensor_tensor(out=xn[:], in0=xn[:], in1=bnb[:, 0:D],
                                        op=mybir.AluOpType.add)

                if layer == 1:
                    # elu = max(x,0) + exp(min(x,0)) - 1
                    ea = wp.tile([P, D], F32, tag="ea", name="ea", bufs=2)
                    nc.vector.tensor_scalar_max(ea[:], xn[:], 0.0)
                    eb = wp.tile([P, D], F32, tag="eb", name="eb", bufs=2)
                    nc.vector.tensor_scalar_min(eb[:], xn[:], 0.0)
                    nc.scalar.activation(eb[:], eb[:], mybir.ActivationFunctionType.Exp)
                    nc.vector.tensor_tensor(out=ea[:], in0=ea[:], in1=eb[:],
                                            op=mybir.AluOpType.add)
                    nc.vector.tensor_scalar_add(ea[:], ea[:], -1.0)
                    for t4 in range(H):
                        tps = pp.tile([P, P], F32, tag="tp", name="tps")
                        nc.tensor.transpose(tps[:], ea[:, t4 * P:(t4 + 1) * P], ident[:])
                        tcp = wp.tile([P, P], F32, tag="tcp", name="tcp")
                        nc.vector.tensor_copy(tcp[:], tps[:])
                        nc.sync.dma_start(
                            out=hT_loc[t4 * P:(t4 + 1) * P, w * P:(w + 1) * P],
                            in_=tcp[:])
                else:
                    nc.vector.tensor_scalar_max(xn[:], xn[:], 0.0)
                    tps = pp.tile([P, P], F32, tag="tp", name="tps")
                    nc.tensor.transpose(tps[:], xn[:], ident[:])
                    tcp = wp.tile([P, P], F32, tag="tcp", name="tcp")
                    nc.vector.tensor_copy(tcp[:], tps[:])
                    pj = pp.tile([P, H], F32, tag="small", name="pj")
                    nc.tensor.matmul(pj[:, 0:2], lhsT=tcp[:], rhs=vwT[:, :],
                                     start=True, stop=True)
                    ov = wp.tile([P, 2], F32, tag="ov", name="ov")
                    nc.vector.tensor_tensor(out=ov[:], in0=pj[:, 0:2], in1=vbb[:],
                                            op=mybir.AluOpType.add)
                    nc.sync.dma_start(out=outs["out_rows"][w * P:(w + 1) * P, :],
                                      in_=ov[:])

        # ---- program ----
        dense(X1[:], A1[:],
              lambda k, t: ins["embedT"][k * P:(k + 1) * P, t * P:(t + 1) * P],
              r1_sb, bias1)
        edge(1, X1[:], A1[:], ins["df1"], ins["sf1"], ins["srel1"])
        nc.gpsimd.collective_compute(
            "AllGather", mybir.AluOpType.bypass,
            replica_groups=[[g * cfg.QPG + i for i in range(cfg.QPG)]
                            for g in range(cfg.B)],
            ins=[hT_loc[:]], outs=[hT_ag[:]])

        def l2_lhs(k, t):
            q, c = divmod(t, WPC)
            return hT_ag[q * TBL + k * P: q * TBL + (k + 1) * P,
                         c * P:(c + 1) * P]

        dense(X2[:], A2[:], l2_lhs, r2_sb, bias2)
        edge(2, X2[:], A2[:], ins["df2"], ins["sf2"], ins["srel2"])


# --------------------------------------------------------------------------
# runner
# --------------------------------------------------------------------------

_NC_CACHE = {}


def _build_nc(cfg, in_shapes):
    key = (cfg.N, cfg.E, cfg.SLOTS)
    if key in _NC_CACHE:
        return _NC_CACHE[key]
    import concourse.bacc as bacc
    nc = bacc.Bacc("TRN2", target_bir_lowering=False, debug=False,
                   num_devices=8)
    ins = {}
    for name, (shape, dt) in in_shapes.items():
        ins[name] = nc.dram_tensor(name, list(shape), dt, kind="ExternalInput").ap()
    outs = {"out_rows": nc.dram_tensor("out_rows", [cfg.NPC, 2], F32,
                                       kind="ExternalOutput").ap()}
    with tile.TileContext(nc) as tc:
        build_gat(tc, outs, ins, cfg)
    nc.compile()
    _NC_CACHE[key] = nc
    return nc


def run_gat(in_maps, cfg, **kw):
    from concourse import bass_utils
    in_shapes = {k: (v.shape, mybir.dt.from_np(v.dtype))
                 for k, v in in_maps[0].items()}
    nc = _build_nc(cfg, in_shapes)
    res = bass_utils.run_bass_kernel_spmd(nc, in_maps, core_ids=list(range(8)),
                                          **kw)
    return nc, res


def kernel(edge_in, edge_out, embed, W1, b1, a1, g1, bn1,
           W2, b2, a2, g2, bn2, Vw, Vb):
    cfg = FULL
    in_maps = prep_inputs(edge_in, edge_out, embed, W1, b1, a1, g1, bn1,
                          W2, b2, a2, g2, bn2, Vw, Vb, cfg)
    _, res = run_gat(in_maps, cfg)
    row = cfg.N - 1
    q, local = divmod(row, cfg.NPC)
    out = np.stack([res.results[b * cfg.QPG + q]["out_rows"][local]
                    for b in range(cfg.B)])
    return out.astype(np.float32)
